# revision 5
# baseline (speedup 1.0000x reference)
"""Trainium2 Bass kernel for nn_NonFirstLayerAggregator (GNN message passing).

Strategy:
  - Data-parallel over batch B across 8 cores; embedding tables replicated.
  - Host fuses E_bal|E_unbal into one [N+1, 128] table (last row zeros) so each
    gathered neighbor is one 512B contiguous DMA descriptor serving both tables.
  - Invalid neighbor slots (k >= len) are redirected to the zero row on host, so
    the device does a plain gather (no bounds checks, no memsets) followed by a
    strided reduce over the K axis and a per-row scale by 1/max(len,1).
  - Per batch row the kernel gathers 21 slots (10 pos, 10 neg, 1 self) of 128
    floats, reduces pos/neg groups, scales, and assembles the two 192-wide
    outputs.
"""

import numpy as np

B = 100000
K = 10
N = 1000000
D = 64
NCORES = 8
P = 128
NSLOT = 2 * K + 1  # pos 0..9 | neg 10..19 | self 20
DW = 2 * D  # fused row width (bal | unbal)

BC = B // NCORES  # 12500 rows per core
TILES = -(-BC // P)  # 98
BCP = TILES * P  # 12544 (padded per-core batch)

_NC_CACHE = {}


def _build_bass(n_rows, n_tiles, n_table):
    """Build the per-core Bass program. n_rows = padded per-core batch,
    n_tiles = n_rows // P, n_table = embedding rows incl. zero row."""
    from contextlib import ExitStack

    import concourse.bacc as bacc
    import concourse.bass as bass
    import concourse.tile as tile
    from concourse import mybir

    nc = bacc.Bacc("TRN2", target_bir_lowering=False, debug=False)

    idx_d = nc.dram_tensor("idx", [n_rows, NSLOT], mybir.dt.int32, kind="ExternalInput")
    rec_d = nc.dram_tensor("rec", [n_rows, 2], mybir.dt.float32, kind="ExternalInput")
    et_d = nc.dram_tensor("etab", [n_table, DW], mybir.dt.float32, kind="ExternalInput")
    ob_d = nc.dram_tensor(
        "out_bal", [n_rows, 3 * D], mybir.dt.float32, kind="ExternalOutput"
    )
    ou_d = nc.dram_tensor(
        "out_unbal", [n_rows, 3 * D], mybir.dt.float32, kind="ExternalOutput"
    )

    with tile.TileContext(nc) as tc:
        with ExitStack() as ctx:
            io_pool = ctx.enter_context(tc.tile_pool(name="io", bufs=4))
            g_pool = ctx.enter_context(tc.tile_pool(name="g", bufs=3))
            s_pool = ctx.enter_context(tc.tile_pool(name="s", bufs=3))
            o_pool = ctx.enter_context(tc.tile_pool(name="o", bufs=3))

            for t in range(n_tiles):
                r0 = t * P
                idx_sb = io_pool.tile([P, NSLOT], mybir.dt.int32)
                nc.sync.dma_start(out=idx_sb[:], in_=idx_d.ap()[r0 : r0 + P, :])
                rec_sb = io_pool.tile([P, 2], mybir.dt.float32)
                nc.sync.dma_start(out=rec_sb[:], in_=rec_d.ap()[r0 : r0 + P, :])

                g = g_pool.tile([P, NSLOT, DW], mybir.dt.float32)
                # HW constraint: indirect DMA consumes one index per dest
                # partition, so gather each of the 21 slots separately.
                for j in range(NSLOT):
                    nc.gpsimd.indirect_dma_start(
                        out=g[:, j, :],
                        out_offset=None,
                        in_=et_d.ap(),
                        in_offset=bass.IndirectOffsetOnAxis(
                            ap=idx_sb[:, j : j + 1], axis=0
                        ),
                    )

                s_pos = s_pool.tile([P, DW], mybir.dt.float32)
                nc.vector.reduce_sum(
                    out=s_pos[:],
                    in_=g[:, 0:K, :].rearrange("p k d -> p d k"),
                    axis=mybir.AxisListType.X,
                )
                s_neg = s_pool.tile([P, DW], mybir.dt.float32)
                nc.vector.reduce_sum(
                    out=s_neg[:],
                    in_=g[:, K : 2 * K, :].rearrange("p k d -> p d k"),
                    axis=mybir.AxisListType.X,
                )

                ob = o_pool.tile([P, 3 * D], mybir.dt.float32)
                ou = o_pool.tile([P, 3 * D], mybir.dt.float32)
                nc.vector.tensor_scalar_mul(ob[:, 0:D], s_pos[:, 0:D], rec_sb[:, 0:1])
                nc.vector.tensor_scalar_mul(ou[:, 0:D], s_pos[:, D:DW], rec_sb[:, 0:1])
                nc.vector.tensor_scalar_mul(
                    ou[:, D : 2 * D], s_neg[:, 0:D], rec_sb[:, 1:2]
                )
                nc.vector.tensor_scalar_mul(
                    ob[:, D : 2 * D], s_neg[:, D:DW], rec_sb[:, 1:2]
                )
                nc.scalar.copy(ob[:, 2 * D : 3 * D], g[:, 2 * K, 0:D])
                nc.scalar.copy(ou[:, 2 * D : 3 * D], g[:, 2 * K, D:DW])

                nc.sync.dma_start(out=ob_d.ap()[r0 : r0 + P, :], in_=ob[:])
                nc.sync.dma_start(out=ou_d.ap()[r0 : r0 + P, :], in_=ou[:])

    nc.compile()
    nc.finalize()
    return nc


def _get_nc(n_rows, n_tiles, n_table):
    key = (n_rows, n_tiles, n_table)
    if key not in _NC_CACHE:
        _NC_CACHE[key] = _build_bass(n_rows, n_tiles, n_table)
    return _NC_CACHE[key]


def _prepare_host(nodes, neigh_pos, neigh_neg, len_pos, len_neg, E_bal, E_unbal):
    """Fused table + masked indices + reciprocal lengths."""
    n = E_bal.shape[0]
    etab = np.empty((n + 1, DW), np.float32)
    etab[:n, :D] = E_bal
    etab[:n, D:] = E_unbal
    etab[n] = 0.0

    b = nodes.shape[0]
    ar = np.arange(K, dtype=np.int32)[None, :]
    idx = np.empty((b, NSLOT), np.int32)
    idx[:, 0:K] = np.where(ar < len_pos[:, None], neigh_pos, n)
    idx[:, K : 2 * K] = np.where(ar < len_neg[:, None], neigh_neg, n)
    idx[:, 2 * K] = nodes

    rec = np.empty((b, 2), np.float32)
    rec[:, 0] = 1.0 / np.maximum(len_pos, 1).astype(np.float32)
    rec[:, 1] = 1.0 / np.maximum(len_neg, 1).astype(np.float32)
    return etab, idx, rec


def kernel(nodes, neigh_pos, neigh_neg, len_pos, len_neg, E_bal, E_unbal):
    from concourse.bass_utils import run_bass_kernel_spmd

    nodes = np.asarray(nodes)
    neigh_pos = np.asarray(neigh_pos)
    neigh_neg = np.asarray(neigh_neg)
    len_pos = np.asarray(len_pos)
    len_neg = np.asarray(len_neg)
    E_bal = np.asarray(E_bal)
    E_unbal = np.asarray(E_unbal)

    n = E_bal.shape[0]
    etab, idx, rec = _prepare_host(
        nodes, neigh_pos, neigh_neg, len_pos, len_neg, E_bal, E_unbal
    )

    in_maps = []
    for c in range(NCORES):
        sl = slice(c * BC, (c + 1) * BC)
        idx_c = np.full((BCP, NSLOT), n, np.int32)
        idx_c[:BC] = idx[sl]
        rec_c = np.ones((BCP, 2), np.float32)
        rec_c[:BC] = rec[sl]
        in_maps.append({"idx": idx_c, "rec": rec_c, "etab": etab})

    nc = _get_nc(BCP, TILES, n + 1)
    res = run_bass_kernel_spmd(nc, in_maps, core_ids=list(range(NCORES)))
    ob = np.concatenate([res.results[c]["out_bal"][:BC] for c in range(NCORES)], axis=0)
    ou = np.concatenate(
        [res.results[c]["out_unbal"][:BC] for c in range(NCORES)], axis=0
    )
    return ob, ou
